# revision 27
# baseline (speedup 1.0000x reference)
"""Trainium2 Bass kernel for nn_CrossAttnActGPT2Attention.

Math: the module is cross-attention from S=4096 query tokens to a KV
sequence of length 2 (a learned no-op token and one token projected from
`activation`).  Softmax over 2 keys is a sigmoid of the score difference,
so the whole module folds, per batch element b, into

    out[s, :] = c + sigmoid(x[s, :] @ G_b + e_b) @ U_b

with
    G_b[:, h] = W_q[:, h*64:(h+1)*64] @ (k1_b[h] - k0[h])      [D, H]
    e_b[h]    = b_q[h*64:(h+1)*64] . (k1_b[h] - k0[h])         [H]
    U_b[h, :] = (v1_b[h] - v0[h]) @ W_proj[h*64:(h+1)*64, :]   [H, D]
    c         = v0.flatten() @ W_proj + b_proj                 [D]
    (k1_b, v1_b from kv = activation[b] @ W_kv + b_kv; k0, v0 = no-op token)

This is exact (validated to ~8e-7 rel. Frobenius error vs the f32 jax
reference).  The per-batch G/U/e/c precompute is ~34 MFLOP and runs on
host; the device kernel streams x in and the output out -- the
memory-bound part -- sharded data-parallel, one batch element per core.

Memory-bound, so the streamed tensors are compressed: x is sent as fp8
e3m4 (4 MiB/core) and the output is written back as bf16 (8 MiB/core,
upcast on host); G/U/sig run in bf16 with f32 PSUM accumulation
(~1.1e-2 rel Frobenius total, under the 2e-2 gate).  DMA is issued
from several engines (SP for x, Pool/SP for outputs, PE for the small
constants) so no single queue serializes the traffic.

Device kernel per core, per s-block of 512 (4 subtiles of 128):
  mm1 (flipped): pd^T[s', t*32+h] = sum_c x_chunk[c][:, s']^T @ G[c][:, h]
      - x chunk [128, 128] is the STATIONARY operand, G [128, 17] moving
        (16 heads + 1 zero "homogeneous" column), so each matmul streams
        only 17 columns.  A K=1 matmul (ones^T @ e_pack) with start=True
        initializes the PSUM bank with the per-(t,h) sigmoid bias first
        (e_pack also carries bias 30 in each hom slot -> sigmoid == 1,
        which later multiplies U's last row = c); the 32 x-matmuls then
        accumulate on top (start=False, sub-bank groups).
  sigmoid (ACT): sig_sT[128, 4x32] = sigmoid(pd^T), bf16
  transpose (PE): sig_hT[t*32+h, s'] = sig_sT^T via identity matmul
  mm2: out[t-rows, half] = sig_hT[32t:32t+17, :]^T @ Uq[32t:32t+17, half]
      (tile_position=(32t, 0)); PSUM -> SBUF bf16 wide copies (ACT/DVE),
      then DMA out (Pool/SP).
"""

import ml_dtypes
import numpy as np

import concourse.bass as bass
import concourse.tile as tile
from concourse import mybir
from concourse.bass_utils import run_bass_kernel_spmd
from concourse.vector_clock import ScopedClock

B, S, D, H, HD = 8, 4096, 1024, 16, 64
SBLK = 512           # s-columns per block
NBLK = S // SBLK     # 8
NSUB = SBLK // 128   # 4 subtiles per block
NCHUNK = D // 128    # 8 contraction chunks
HP = H + 1           # 16 heads + homogeneous channel
F32 = mybir.dt.float32

DT_X = mybir.dt.float8e3      # x (mm1 stationary; 1 B/elem on the wire)
DT_G = mybir.dt.bfloat16      # G (mm1 moving operand)
DT_SU = mybir.dt.bfloat16     # sig / U / identity / e_pack
DT_O = mybir.dt.bfloat16      # output staging + HBM writeback

NP_X = ml_dtypes.float8_e3m4
NP_BF16 = ml_dtypes.bfloat16


class _TileContextSplitDrain(tile.TileContext):
    """The walrus build here rejects >1 sync wait on a CTRL (drain)
    instruction; split the final drain's waits across single-wait NOPs."""

    def _drain_and_barrier(self, tick_clock, wait_clock):
        nc = self.nc
        probe = nc.sync.nop(nofuse=True, hint="drain_wait_probe")
        wait_clock.add_sem_waits(
            probe.ins, ScopedClock({None: tick_clock.global_clock})
        )
        si = probe.ins.sync_info
        waits = list(si.on_wait or []) if si is not None else []
        if len(waits) > 1:
            si.on_wait = [waits[0]]
            for w in waits[1:]:
                extra = nc.sync.nop(nofuse=True, hint="drain_wait_split")
                extra.ins.sync_info = type(si)(on_wait=[w], on_update=[])
        nc.sync.drain()
        nc.all_engine_barrier()
        assert self.sems is not None
        popped = nc._tile_sem_poison_stack.pop()
        assert popped is self._sem_poison
        nc.clear_and_free_semaphores(list(self.sems.allocated().values()))
        nc.all_engine_barrier()


def _split_multi_waits(nc):
    """Walrus here allows at most one sync-wait per instruction.  Move
    extra waits of any instruction onto same-engine NOPs placed directly
    before it (same sequencer => identical blocking semantics)."""
    n_split = 0
    for bb in nc.main_func.blocks:
        insts = list(bb.instructions)
        new_list = []
        changed = False
        for inst in insts:
            si = inst.sync_info
            waits = list(si.on_wait) if (si is not None and si.on_wait) else []
            if len(waits) > 1:
                changed = True
                for k, w in enumerate(waits[:-1]):
                    nop = mybir.InstNoOp(
                        name=f"{inst.name}-ws{k}", ins=[], outs=[]
                    )
                    nop.engine = inst.engine
                    nop.sync_info = type(si)(on_wait=[w], on_update=[])
                    nc.register_instruction(nop)
                    new_list.append(nop)
                    n_split += 1
                si.on_wait = [waits[-1]]
            new_list.append(inst)
        if changed:
            bb.instructions = new_list
    return n_split


def _build_kernel():
    nc = bass.Bass("TRN2", target_bir_lowering=False, debug=False, num_devices=B)

    xT = nc.dram_tensor("xT", [D, S], DT_X, kind="ExternalInput")
    G = nc.dram_tensor("G", [D, HP], DT_G, kind="ExternalInput")
    U = nc.dram_tensor("U", [128, D], DT_SU, kind="ExternalInput")
    e = nc.dram_tensor("e", [1, 128], DT_SU, kind="ExternalInput")
    ident = nc.dram_tensor("ident", [128, 128], DT_SU, kind="ExternalInput")
    out = nc.dram_tensor("out", [S, D], DT_O, kind="ExternalOutput")

    # [D, S] -> [p, chunk, s];  [S, D] -> [p, subtile, j]
    xT_v = xT.ap().rearrange("(c p) s -> p c s", p=128)
    out_v = out.ap().rearrange("(t p) j -> p t j", p=128)

    # 256-column first/last blocks shorten the pipeline fill and drain;
    # (subtile offset, subtile count) per block
    BLOCKS = [(4 * i, 4) for i in range(8)]

    with _TileContextSplitDrain(nc) as tc:
        with (
            tc.tile_pool(name="singles", bufs=1) as singles,
            tc.tile_pool(name="xt", bufs=6) as xt_pool,
            tc.tile_pool(name="sigst", bufs=2) as sigst_pool,
            tc.tile_pool(name="sight", bufs=2) as sight_pool,
            tc.tile_pool(name="osb", bufs=3) as out_pool,
            tc.tile_pool(name="pdpt", bufs=2, space="PSUM") as pdpt_pool,
            tc.tile_pool(name="po", bufs=3, space="PSUM") as po_pool,
        ):
            g_sb = singles.tile([128, NCHUNK, HP], DT_G)
            u_sb = singles.tile([128, D], DT_SU)
            e_sb = singles.tile([1, 128], DT_SU)
            ones_sb = singles.tile([1, 128], DT_SU)
            id_sb = singles.tile([128, 128], DT_SU)
            # constants split over the startup-idle queues (ACT and
            # Pool); a warm-up sigmoid on a const AP prepays the 1283ns
            # activation-table load while the first x block is in flight
            warm_sb = singles.tile([1, 1], DT_SU)
            nc.scalar.dma_start(out=g_sb, in_=G.ap().rearrange("(c p) h -> p c h", p=128))
            nc.scalar.dma_start(out=e_sb, in_=e.ap())
            nc.scalar.activation(out=warm_sb[:, :],
                                 in_=nc.const_aps.tensor(0.0, (1, 1)),
                                 func=mybir.ActivationFunctionType.Sigmoid)
            nc.gpsimd.memset(ones_sb[:, :], 1.0)
            nc.gpsimd.dma_start(out=id_sb, in_=ident.ap())
            nc.gpsimd.dma_start(out=u_sb, in_=U.ap())

            state = {}

            def stage_mm1(bi):
                st0, nt = BLOCKS[bi]
                ncol = 128 * nt
                # two half-tiles per block: halves the DMA grain so the
                # first chunks land sooner and SP stays interleavable
                xt_a = xt_pool.tile([128, NCHUNK // 2, SBLK], DT_X)
                xt_b = xt_pool.tile([128, NCHUNK // 2, SBLK], DT_X)
                s0 = st0 * 128
                nc.sync.dma_start(out=xt_a[:, :, 0:ncol], in_=xT_v[:, 0:4, s0:s0 + ncol])
                nc.sync.dma_start(out=xt_b[:, :, 0:ncol], in_=xT_v[:, 4:8, s0:s0 + ncol])
                # pd (sigmoid input, cols 0:128) and the transpose target
                # (cols 128:256) share one PSUM bank: every producer below
                # re-marks the bank's zero region with start=True before
                # writing, so sub-bank groups coexist (skip_group_check)
                pdpt = pdpt_pool.tile([128, 256], F32)
                nc.tensor.matmul(
                    pdpt[:, 0:32 * nt],
                    ones_sb[:, :],
                    e_sb[:, 0:32 * nt],
                    start=True, stop=False, skip_group_check=True,
                )
                for c in range(NCHUNK):
                    xt_h = (xt_a, xt_b)[c // 4]
                    for t in range(nt):
                        nc.tensor.matmul(
                            pdpt[:, 32 * t:32 * t + HP],
                            xt_h[:, c % 4, t * 128:(t + 1) * 128],
                            g_sb[:, c, :],
                            start=False,
                            stop=(t == nt - 1 and c == NCHUNK - 1),
                            skip_group_check=True,
                        )
                state[bi] = {"pdpt": pdpt, "nt": nt, "st0": st0}

            def stage_sig(bi):
                st = state[bi]
                nt = st["nt"]
                sig_st = sigst_pool.tile([128, 128], DT_SU)
                nc.scalar.activation(
                    out=sig_st[:, 0:32 * nt],
                    in_=st["pdpt"][:, 0:32 * nt],
                    func=mybir.ActivationFunctionType.Sigmoid,
                )
                st["sig_st"] = sig_st

            def stage_mm2(bi):
                st = state[bi]
                nt = st["nt"]
                sig_ht = st["sig_ht"]
                osb = out_pool.tile([128, NSUB, D], DT_O)
                pos = []
                for t in range(nt):
                    po = po_pool.tile([128, 2, 512], F32)
                    pos.append(po)
                    for half in range(2):
                        nc.tensor.matmul(
                            po[:, half, :],
                            sig_ht[32 * t:32 * t + HP, :],
                            u_sb[32 * t:32 * t + HP,
                                 half * 512:(half + 1) * 512],
                            start=True,
                            stop=True,
                            tile_position=(32 * t, 0),
                        )
                st["osb"] = osb
                st["pos"] = pos

            def stage_transpose(bi):
                # transpose via plain matmul: sig_st.T @ I (the is_transpose
                # path trips the interpreter's permutation check during the
                # tile scheduling pass, which runs on dummy data)
                st = state[bi]
                nt = st["nt"]
                nc.tensor.matmul(st["pdpt"][0:32 * nt, 128:256],
                                 st["sig_st"][:, 0:32 * nt],
                                 id_sb[:, :],
                                 start=True, stop=True, skip_group_check=True)

            def stage_copies(bi, last=False):
                # wide f32->bf16 copies, interleaved ACT/DVE so each engine's
                # first copy is the earliest-ready PSUM pair; the final block
                # uses narrow copies so its writeback can start sooner
                st = state[bi]
                nt = st["nt"]
                osb, pos = st["osb"], st["pos"]
                if last:
                    nc.scalar.copy(osb[:, 0, :], pos[0][:, :, :])
                    nc.vector.tensor_copy(osb[:, 2, :], pos[2][:, :, :])
                    nc.scalar.copy(osb[:, 1, :], pos[1][:, :, :])
                    nc.vector.tensor_copy(osb[:, 3, :], pos[3][:, :, :])
                elif nt == 2:
                    nc.scalar.copy(osb[:, 0, :], pos[0][:, :, :])
                    nc.vector.tensor_copy(osb[:, 1, :], pos[1][:, :, :])
                else:
                    nc.scalar.copy(osb[:, 0, :], pos[0][:, :, :])
                    nc.vector.tensor_copy(osb[:, 2, :], pos[2][:, :, :])
                    nc.scalar.copy(osb[:, 1, :], pos[1][:, :, :])
                    nc.vector.tensor_copy(osb[:, 3, :], pos[3][:, :, :])

            def stage_sigt(bi):
                st = state[bi]
                nt = st["nt"]
                sig_ht = sight_pool.tile([128, 128], DT_SU)
                nc.vector.tensor_copy(sig_ht[0:32 * nt, :],
                                      st["pdpt"][0:32 * nt, 128:256])
                st["sig_ht"] = sig_ht

            def stage_out(bi, last=False):
                st = state.pop(bi)
                nt, st0 = st["nt"], st["st0"]
                osb = st["osb"]
                if last:
                    # tail: per-subtile writeback spread over all DMA queues
                    # so the drain isn't gated on one engine
                    engs = (nc.gpsimd, nc.sync, nc.scalar, nc.gpsimd)
                    for t in range(nt):
                        engs[t].dma_start(
                            out=out_v[:, st0 + t:st0 + t + 1, :],
                            in_=osb[:, t:t + 1, :])
                elif nt == 2:
                    nc.gpsimd.dma_start(out=out_v[:, st0:st0 + 1, :], in_=osb[:, 0:1, :])
                    nc.sync.dma_start(out=out_v[:, st0 + 1:st0 + 2, :], in_=osb[:, 1:2, :])
                else:
                    nc.gpsimd.dma_start(out=out_v[:, st0:st0 + 3, :], in_=osb[:, 0:3, :])
                    nc.sync.dma_start(out=out_v[:, st0 + 3:st0 + 4, :], in_=osb[:, 3:4, :])

            nb = len(BLOCKS)
            for bi in range(nb):
                stage_mm1(bi)         # SP dma; PE e-MM + x-MMs
                stage_sig(bi)         # ACT (ahead of prior-block copies)
                if bi > 0:
                    stage_mm2(bi - 1)     # PE (fills the sigmoid latency)
                stage_transpose(bi)   # PE (after po-MMs in engine order)
                stage_sigt(bi)        # DVE; parks in the wait queue until
                                      # the transpose lands, copies flow OOO
                if bi > 0:
                    stage_copies(bi - 1)  # ACT/DVE
                    stage_out(bi - 1)     # Pool + SP
            stage_mm2(nb - 1)
            stage_copies(nb - 1, last=True)
            stage_out(nb - 1, last=True)

    _split_multi_waits(nc)
    return nc


_NC_CACHE = None


def _get_nc():
    global _NC_CACHE
    if _NC_CACHE is None:
        _NC_CACHE = _build_kernel()
    return _NC_CACHE


def _host_precompute(activation, W_q, b_q, W_kv, b_kv, no_op_k, no_op_v,
                     W_proj, b_proj):
    """Per-batch G [B,D,H], U' [B,H+1,D] (last row = c), e [B,H,1] in f64."""
    act = activation.astype(np.float64)
    W_q = W_q.astype(np.float64)
    b_q = b_q.astype(np.float64)
    W_kv = W_kv.astype(np.float64)
    b_kv = b_kv.astype(np.float64)
    k0 = no_op_k.astype(np.float64).reshape(H, HD)
    v0 = no_op_v.astype(np.float64).reshape(H, HD)
    W_p = W_proj.astype(np.float64)
    b_p = b_proj.astype(np.float64)

    kv = act @ W_kv + b_kv
    k1 = kv[:, :D].reshape(B, H, HD)
    v1 = kv[:, D:].reshape(B, H, HD)
    dk = k1 - k0[None]
    dv = v1 - v0[None]
    G = np.einsum("dhe,bhe->bdh", W_q.reshape(D, H, HD), dk)
    e = np.einsum("he,bhe->bh", b_q.reshape(H, HD), dk)
    U = np.einsum("bhe,hej->bhj", dv, W_p.reshape(H, HD, D))
    c = v0.reshape(-1) @ W_p + b_p
    Up = np.concatenate([U, np.broadcast_to(c, (B, 1, D))], axis=1)
    # 17-channel packing: 16 heads + a homogeneous channel whose zero G
    # column and bias 30 make sigmoid == 1.0, multiplying U's last row
    # (= c) in mm2.  U and e replicate at partition offsets 0/32/64/96
    # (mm2 row-tiles 4-way; e_pack is laid out [t*32 + h]).
    G17 = np.zeros((B, D, HP))
    G17[:, :, :H] = G
    Uq = np.zeros((B, 128, D))
    eq = np.zeros((B, 128))
    for j in range(4):
        Uq[:, 32 * j:32 * j + HP] = Up
        eq[:, 32 * j:32 * j + H] = e
        eq[:, 32 * j + H] = 30.0
    return G17.astype(np.float32), Uq.astype(np.float32), eq.astype(np.float32)


def _prep_core_inputs(hidden_states, activation, W_q, b_q, W_kv, b_kv,
                      no_op_k, no_op_v, W_proj, b_proj):
    """Per-core input dicts in the wire dtypes the device kernel expects."""
    G17, Uq, eq = _host_precompute(activation, W_q, b_q, W_kv, b_kv,
                                   no_op_k, no_op_v, W_proj, b_proj)
    Gb = G17.astype(NP_BF16)
    Ub = Uq.astype(NP_BF16)
    x8 = np.asarray(hidden_states, dtype=np.float32).astype(NP_X)
    ident = np.eye(128, dtype=np.float32).astype(NP_BF16)
    eb = eq.astype(NP_BF16)
    return [
        {"xT": np.ascontiguousarray(x8[b].T),
         "G": np.ascontiguousarray(Gb[b]),
         "U": np.ascontiguousarray(Ub[b]),
         "e": np.ascontiguousarray(eb[b].reshape(1, 128)),
         "ident": ident}
        for b in range(B)
    ]


def kernel(hidden_states, activation, W_q, b_q, W_kv, b_kv, no_op_k, no_op_v,
           W_proj, b_proj):
    hidden_states = np.asarray(hidden_states)
    activation = np.asarray(activation)
    W_q, b_q = np.asarray(W_q), np.asarray(b_q)
    W_kv, b_kv = np.asarray(W_kv), np.asarray(b_kv)
    no_op_k, no_op_v = np.asarray(no_op_k), np.asarray(no_op_v)
    W_proj, b_proj = np.asarray(W_proj), np.asarray(b_proj)
    in_maps = _prep_core_inputs(hidden_states, activation, W_q, b_q, W_kv,
                                b_kv, no_op_k, no_op_v, W_proj, b_proj)
    nc = _get_nc()
    res = run_bass_kernel_spmd(nc, in_maps, core_ids=list(range(B)))
    return np.stack(
        [np.asarray(res.results[b]["out"]).astype(np.float32) for b in range(B)],
        axis=0,
    )


# revision 28
# speedup vs baseline: 1.0223x; 1.0223x over previous
"""Trainium2 Bass kernel for nn_CrossAttnActGPT2Attention.

Math: the module is cross-attention from S=4096 query tokens to a KV
sequence of length 2 (a learned no-op token and one token projected from
`activation`).  Softmax over 2 keys is a sigmoid of the score difference,
so the whole module folds, per batch element b, into

    out[s, :] = c + sigmoid(x[s, :] @ G_b + e_b) @ U_b

with
    G_b[:, h] = W_q[:, h*64:(h+1)*64] @ (k1_b[h] - k0[h])      [D, H]
    e_b[h]    = b_q[h*64:(h+1)*64] . (k1_b[h] - k0[h])         [H]
    U_b[h, :] = (v1_b[h] - v0[h]) @ W_proj[h*64:(h+1)*64, :]   [H, D]
    c         = v0.flatten() @ W_proj + b_proj                 [D]
    (k1_b, v1_b from kv = activation[b] @ W_kv + b_kv; k0, v0 = no-op token)

This is exact (validated to ~8e-7 rel. Frobenius error vs the f32 jax
reference).  The per-batch G/U/e/c precompute is ~34 MFLOP and runs on
host; the device kernel streams x in and the output out -- the
memory-bound part -- sharded data-parallel, one batch element per core.

Memory-bound, so the streamed tensors are compressed: x is sent as fp8
e3m4 (4 MiB/core) and the output is written back as bf16 (8 MiB/core,
upcast on host); G/U/sig run in bf16 with f32 PSUM accumulation
(~1.1e-2 rel Frobenius total, under the 2e-2 gate).  DMA is issued
from several engines (SP for x, Pool/SP for outputs, PE for the small
constants) so no single queue serializes the traffic.

Device kernel per core, per s-block of 512 (4 subtiles of 128):
  mm1 (flipped): pd^T[s', t*32+h] = sum_c x_chunk[c][:, s']^T @ G[c][:, h]
      - x chunk [128, 128] is the STATIONARY operand, G [128, 17] moving
        (16 heads + 1 zero "homogeneous" column), so each matmul streams
        only 17 columns.  A K=1 matmul (ones^T @ e_pack) with start=True
        initializes the PSUM bank with the per-(t,h) sigmoid bias first
        (e_pack also carries bias 30 in each hom slot -> sigmoid == 1,
        which later multiplies U's last row = c); the 32 x-matmuls then
        accumulate on top (start=False, sub-bank groups).
  sigmoid (ACT): sig_sT[128, 4x32] = sigmoid(pd^T), bf16
  transpose (PE): sig_hT[t*32+h, s'] = sig_sT^T via identity matmul
  mm2: out[t-rows, half] = sig_hT[32t:32t+17, :]^T @ Uq[32t:32t+17, half]
      (tile_position=(32t, 0)); PSUM -> SBUF bf16 wide copies (ACT/DVE),
      then DMA out (Pool/SP).
"""

import ml_dtypes
import numpy as np

import concourse.bass as bass
import concourse.tile as tile
from concourse import mybir
from concourse.bass_utils import run_bass_kernel_spmd
from concourse.vector_clock import ScopedClock

B, S, D, H, HD = 8, 4096, 1024, 16, 64
SBLK = 512           # s-columns per block
NBLK = S // SBLK     # 8
NSUB = SBLK // 128   # 4 subtiles per block
NCHUNK = D // 128    # 8 contraction chunks
HP = H + 1           # 16 heads + homogeneous channel
F32 = mybir.dt.float32

DT_X = mybir.dt.float8e3      # x (mm1 stationary; 1 B/elem on the wire)
DT_G = mybir.dt.bfloat16      # G (mm1 moving operand)
DT_SU = mybir.dt.bfloat16     # sig / U / identity / e_pack
DT_O = mybir.dt.bfloat16      # output staging + HBM writeback

NP_X = ml_dtypes.float8_e3m4
NP_BF16 = ml_dtypes.bfloat16


class _TileContextSplitDrain(tile.TileContext):
    """The walrus build here rejects >1 sync wait on a CTRL (drain)
    instruction; split the final drain's waits across single-wait NOPs."""

    def _drain_and_barrier(self, tick_clock, wait_clock):
        nc = self.nc
        probe = nc.sync.nop(nofuse=True, hint="drain_wait_probe")
        wait_clock.add_sem_waits(
            probe.ins, ScopedClock({None: tick_clock.global_clock})
        )
        si = probe.ins.sync_info
        waits = list(si.on_wait or []) if si is not None else []
        if len(waits) > 1:
            si.on_wait = [waits[0]]
            for w in waits[1:]:
                extra = nc.sync.nop(nofuse=True, hint="drain_wait_split")
                extra.ins.sync_info = type(si)(on_wait=[w], on_update=[])
        nc.sync.drain()
        nc.all_engine_barrier()
        assert self.sems is not None
        popped = nc._tile_sem_poison_stack.pop()
        assert popped is self._sem_poison
        nc.clear_and_free_semaphores(list(self.sems.allocated().values()))
        nc.all_engine_barrier()


def _split_multi_waits(nc):
    """Walrus here allows at most one sync-wait per instruction.  Move
    extra waits of any instruction onto same-engine NOPs placed directly
    before it (same sequencer => identical blocking semantics)."""
    n_split = 0
    for bb in nc.main_func.blocks:
        insts = list(bb.instructions)
        new_list = []
        changed = False
        for inst in insts:
            si = inst.sync_info
            waits = list(si.on_wait) if (si is not None and si.on_wait) else []
            if len(waits) > 1:
                changed = True
                for k, w in enumerate(waits[:-1]):
                    nop = mybir.InstNoOp(
                        name=f"{inst.name}-ws{k}", ins=[], outs=[]
                    )
                    nop.engine = inst.engine
                    nop.sync_info = type(si)(on_wait=[w], on_update=[])
                    nc.register_instruction(nop)
                    new_list.append(nop)
                    n_split += 1
                si.on_wait = [waits[-1]]
            new_list.append(inst)
        if changed:
            bb.instructions = new_list
    return n_split


def _build_kernel():
    nc = bass.Bass("TRN2", target_bir_lowering=False, debug=False, num_devices=B)

    xT = nc.dram_tensor("xT", [D, S], DT_X, kind="ExternalInput")
    G = nc.dram_tensor("G", [D, HP], DT_G, kind="ExternalInput")
    U = nc.dram_tensor("U", [128, D], DT_SU, kind="ExternalInput")
    e = nc.dram_tensor("e", [1, 128], DT_SU, kind="ExternalInput")
    ident = nc.dram_tensor("ident", [128, 128], DT_SU, kind="ExternalInput")
    out = nc.dram_tensor("out", [S, D], DT_O, kind="ExternalOutput")

    # [D, S] -> [p, chunk, s];  [S, D] -> [p, subtile, j]
    xT_v = xT.ap().rearrange("(c p) s -> p c s", p=128)
    out_v = out.ap().rearrange("(t p) j -> p t j", p=128)

    # 256-column first/last blocks shorten the pipeline fill and drain;
    # (subtile offset, subtile count) per block
    BLOCKS = [(4 * i, 4) for i in range(8)]

    with _TileContextSplitDrain(nc) as tc:
        with (
            tc.tile_pool(name="singles", bufs=1) as singles,
            tc.tile_pool(name="xt", bufs=6) as xt_pool,
            tc.tile_pool(name="sigst", bufs=2) as sigst_pool,
            tc.tile_pool(name="sight", bufs=2) as sight_pool,
            tc.tile_pool(name="osb", bufs=3) as out_pool,
            tc.tile_pool(name="pdpt", bufs=2, space="PSUM") as pdpt_pool,
            tc.tile_pool(name="po", bufs=3, space="PSUM") as po_pool,
        ):
            g_sb = singles.tile([128, NCHUNK, HP], DT_G)
            u_sb = singles.tile([128, D], DT_SU)
            e_sb = singles.tile([1, 128], DT_SU)
            ones_sb = singles.tile([1, 128], DT_SU)
            id_sb = singles.tile([128, 128], DT_SU)
            # constants split over the startup-idle queues (ACT and
            # Pool); a warm-up sigmoid on a const AP prepays the 1283ns
            # activation-table load while the first x block is in flight
            warm_sb = singles.tile([1, 1], DT_SU)
            nc.scalar.dma_start(out=g_sb, in_=G.ap().rearrange("(c p) h -> p c h", p=128))
            nc.scalar.dma_start(out=e_sb, in_=e.ap())
            nc.scalar.activation(out=warm_sb[:, :],
                                 in_=nc.const_aps.tensor(0.0, (1, 1)),
                                 func=mybir.ActivationFunctionType.Sigmoid)
            nc.gpsimd.memset(ones_sb[:, :], 1.0)
            nc.gpsimd.dma_start(out=id_sb, in_=ident.ap())
            nc.gpsimd.dma_start(out=u_sb, in_=U.ap())

            state = {}

            def stage_mm1(bi):
                st0, nt = BLOCKS[bi]
                ncol = 128 * nt
                # two half-tiles per block: halves the DMA grain so the
                # first chunks land sooner and SP stays interleavable
                xt_a = xt_pool.tile([128, NCHUNK // 2, SBLK], DT_X)
                xt_b = xt_pool.tile([128, NCHUNK // 2, SBLK], DT_X)
                s0 = st0 * 128
                nc.sync.dma_start(out=xt_a[:, :, 0:ncol], in_=xT_v[:, 0:4, s0:s0 + ncol])
                nc.sync.dma_start(out=xt_b[:, :, 0:ncol], in_=xT_v[:, 4:8, s0:s0 + ncol])
                # pd (sigmoid input, cols 0:128) and the transpose target
                # (cols 128:256) share one PSUM bank: every producer below
                # re-marks the bank's zero region with start=True before
                # writing, so sub-bank groups coexist (skip_group_check)
                pdpt = pdpt_pool.tile([128, 256], F32)
                nc.tensor.matmul(
                    pdpt[:, 0:32 * nt],
                    ones_sb[:, :],
                    e_sb[:, 0:32 * nt],
                    start=True, stop=False, skip_group_check=True,
                )
                for c in range(NCHUNK):
                    xt_h = (xt_a, xt_b)[c // 4]
                    for t in range(nt):
                        nc.tensor.matmul(
                            pdpt[:, 32 * t:32 * t + HP],
                            xt_h[:, c % 4, t * 128:(t + 1) * 128],
                            g_sb[:, c, :],
                            start=False,
                            stop=(t == nt - 1 and c == NCHUNK - 1),
                            skip_group_check=True,
                        )
                state[bi] = {"pdpt": pdpt, "nt": nt, "st0": st0}

            def stage_sig(bi):
                st = state[bi]
                nt = st["nt"]
                sig_st = sigst_pool.tile([128, 128], DT_SU)
                nc.scalar.activation(
                    out=sig_st[:, 0:32 * nt],
                    in_=st["pdpt"][:, 0:32 * nt],
                    func=mybir.ActivationFunctionType.Sigmoid,
                )
                st["sig_st"] = sig_st

            def stage_mm2(bi):
                st = state[bi]
                nt = st["nt"]
                sig_ht = st["sig_ht"]
                osb = out_pool.tile([128, NSUB, D], DT_O)
                pos = []
                for t in range(nt):
                    po = po_pool.tile([128, 2, 512], F32)
                    pos.append(po)
                    for half in range(2):
                        nc.tensor.matmul(
                            po[:, half, :],
                            sig_ht[32 * t:32 * t + HP, :],
                            u_sb[32 * t:32 * t + HP,
                                 half * 512:(half + 1) * 512],
                            start=True,
                            stop=True,
                            tile_position=(32 * t, 0),
                        )
                st["osb"] = osb
                st["pos"] = pos

            def stage_transpose(bi):
                # transpose via plain matmul: sig_st.T @ I (the is_transpose
                # path trips the interpreter's permutation check during the
                # tile scheduling pass, which runs on dummy data)
                st = state[bi]
                nt = st["nt"]
                nc.tensor.matmul(st["pdpt"][0:32 * nt, 128:256],
                                 st["sig_st"][:, 0:32 * nt],
                                 id_sb[:, :],
                                 start=True, stop=True, skip_group_check=True)

            def stage_copies(bi, last=False):
                # wide f32->bf16 copies, interleaved ACT/DVE so each engine's
                # first copy is the earliest-ready PSUM pair; the final block
                # uses narrow copies so its writeback can start sooner
                st = state[bi]
                nt = st["nt"]
                osb, pos = st["osb"], st["pos"]
                if last:
                    for half in range(2):
                        for t in range(nt):
                            dst = osb[:, t, half * 512:(half + 1) * 512]
                            if t % 2 == 0:
                                nc.scalar.copy(dst, pos[t][:, half, :])
                            else:
                                nc.vector.tensor_copy(dst, pos[t][:, half, :])
                elif nt == 2:
                    nc.scalar.copy(osb[:, 0, :], pos[0][:, :, :])
                    nc.vector.tensor_copy(osb[:, 1, :], pos[1][:, :, :])
                else:
                    nc.scalar.copy(osb[:, 0, :], pos[0][:, :, :])
                    nc.vector.tensor_copy(osb[:, 2, :], pos[2][:, :, :])
                    nc.scalar.copy(osb[:, 1, :], pos[1][:, :, :])
                    nc.vector.tensor_copy(osb[:, 3, :], pos[3][:, :, :])

            def stage_sigt(bi):
                st = state[bi]
                nt = st["nt"]
                sig_ht = sight_pool.tile([128, 128], DT_SU)
                nc.vector.tensor_copy(sig_ht[0:32 * nt, :],
                                      st["pdpt"][0:32 * nt, 128:256])
                st["sig_ht"] = sig_ht

            def stage_out(bi, last=False):
                st = state.pop(bi)
                nt, st0 = st["nt"], st["st0"]
                osb = st["osb"]
                if last:
                    # tail: per-half writeback spread over all DMA queues so
                    # the drain isn't gated on one engine
                    engs = (nc.gpsimd, nc.sync, nc.scalar)
                    for k, (t, half) in enumerate(
                            (t, h) for h in range(2) for t in range(nt)):
                        j0 = half * 512
                        engs[k % 3].dma_start(
                            out=out_v[:, st0 + t, j0:j0 + 512],
                            in_=osb[:, t, j0:j0 + 512])
                elif nt == 2:
                    nc.gpsimd.dma_start(out=out_v[:, st0:st0 + 1, :], in_=osb[:, 0:1, :])
                    nc.sync.dma_start(out=out_v[:, st0 + 1:st0 + 2, :], in_=osb[:, 1:2, :])
                else:
                    nc.gpsimd.dma_start(out=out_v[:, st0:st0 + 3, :], in_=osb[:, 0:3, :])
                    nc.sync.dma_start(out=out_v[:, st0 + 3:st0 + 4, :], in_=osb[:, 3:4, :])

            nb = len(BLOCKS)
            for bi in range(nb):
                stage_mm1(bi)         # SP dma; PE e-MM + x-MMs
                stage_sig(bi)         # ACT (ahead of prior-block copies)
                if bi > 0:
                    stage_mm2(bi - 1)     # PE (fills the sigmoid latency)
                stage_transpose(bi)   # PE (after po-MMs in engine order)
                stage_sigt(bi)        # DVE; parks in the wait queue until
                                      # the transpose lands, copies flow OOO
                if bi > 0:
                    stage_copies(bi - 1)  # ACT/DVE
                    stage_out(bi - 1)     # Pool + SP
            stage_mm2(nb - 1)
            stage_copies(nb - 1, last=True)
            stage_out(nb - 1, last=True)

    _split_multi_waits(nc)
    return nc


_NC_CACHE = None


def _get_nc():
    global _NC_CACHE
    if _NC_CACHE is None:
        _NC_CACHE = _build_kernel()
    return _NC_CACHE


def _host_precompute(activation, W_q, b_q, W_kv, b_kv, no_op_k, no_op_v,
                     W_proj, b_proj):
    """Per-batch G [B,D,H], U' [B,H+1,D] (last row = c), e [B,H,1] in f64."""
    act = activation.astype(np.float64)
    W_q = W_q.astype(np.float64)
    b_q = b_q.astype(np.float64)
    W_kv = W_kv.astype(np.float64)
    b_kv = b_kv.astype(np.float64)
    k0 = no_op_k.astype(np.float64).reshape(H, HD)
    v0 = no_op_v.astype(np.float64).reshape(H, HD)
    W_p = W_proj.astype(np.float64)
    b_p = b_proj.astype(np.float64)

    kv = act @ W_kv + b_kv
    k1 = kv[:, :D].reshape(B, H, HD)
    v1 = kv[:, D:].reshape(B, H, HD)
    dk = k1 - k0[None]
    dv = v1 - v0[None]
    G = np.einsum("dhe,bhe->bdh", W_q.reshape(D, H, HD), dk)
    e = np.einsum("he,bhe->bh", b_q.reshape(H, HD), dk)
    U = np.einsum("bhe,hej->bhj", dv, W_p.reshape(H, HD, D))
    c = v0.reshape(-1) @ W_p + b_p
    Up = np.concatenate([U, np.broadcast_to(c, (B, 1, D))], axis=1)
    # 17-channel packing: 16 heads + a homogeneous channel whose zero G
    # column and bias 30 make sigmoid == 1.0, multiplying U's last row
    # (= c) in mm2.  U and e replicate at partition offsets 0/32/64/96
    # (mm2 row-tiles 4-way; e_pack is laid out [t*32 + h]).
    G17 = np.zeros((B, D, HP))
    G17[:, :, :H] = G
    Uq = np.zeros((B, 128, D))
    eq = np.zeros((B, 128))
    for j in range(4):
        Uq[:, 32 * j:32 * j + HP] = Up
        eq[:, 32 * j:32 * j + H] = e
        eq[:, 32 * j + H] = 30.0
    return G17.astype(np.float32), Uq.astype(np.float32), eq.astype(np.float32)


def _prep_core_inputs(hidden_states, activation, W_q, b_q, W_kv, b_kv,
                      no_op_k, no_op_v, W_proj, b_proj):
    """Per-core input dicts in the wire dtypes the device kernel expects."""
    G17, Uq, eq = _host_precompute(activation, W_q, b_q, W_kv, b_kv,
                                   no_op_k, no_op_v, W_proj, b_proj)
    Gb = G17.astype(NP_BF16)
    Ub = Uq.astype(NP_BF16)
    x8 = np.asarray(hidden_states, dtype=np.float32).astype(NP_X)
    ident = np.eye(128, dtype=np.float32).astype(NP_BF16)
    eb = eq.astype(NP_BF16)
    return [
        {"xT": np.ascontiguousarray(x8[b].T),
         "G": np.ascontiguousarray(Gb[b]),
         "U": np.ascontiguousarray(Ub[b]),
         "e": np.ascontiguousarray(eb[b].reshape(1, 128)),
         "ident": ident}
        for b in range(B)
    ]


def kernel(hidden_states, activation, W_q, b_q, W_kv, b_kv, no_op_k, no_op_v,
           W_proj, b_proj):
    hidden_states = np.asarray(hidden_states)
    activation = np.asarray(activation)
    W_q, b_q = np.asarray(W_q), np.asarray(b_q)
    W_kv, b_kv = np.asarray(W_kv), np.asarray(b_kv)
    no_op_k, no_op_v = np.asarray(no_op_k), np.asarray(no_op_v)
    W_proj, b_proj = np.asarray(W_proj), np.asarray(b_proj)
    in_maps = _prep_core_inputs(hidden_states, activation, W_q, b_q, W_kv,
                                b_kv, no_op_k, no_op_v, W_proj, b_proj)
    nc = _get_nc()
    res = run_bass_kernel_spmd(nc, in_maps, core_ids=list(range(B)))
    return np.stack(
        [np.asarray(res.results[b]["out"]).astype(np.float32) for b in range(B)],
        axis=0,
    )
